# revision 33
# baseline (speedup 1.0000x reference)
"""Trainium2 Bass kernel for the fused channel-attention block.

Reference computation (per batch item, X = x[n] viewed as [C, HW]):
    t3  = Wp3 @ X                            (1x1 conv, bias 0)
    t4  = sin(pi/2 * X)
    t6  = max(t3, t4)
    lhs = B1 @ X,  rhs = B2 @ X              (grouped 1x1 convs, B* block-diag)
    t5  = lhs @ rhs.T / sqrt(HW)
    t7  = p7w * t5
    out = (t6.T @ t7 / sqrt(C)).T            == t7.T @ t6 / sqrt(C)  in [C, HW]

Algebraic restructuring:
    t5 = B1 @ (X @ X.T) @ B2.T / sqrt(HW)    -- Gram trick: X@X.T is only 256x256
    out[C, HW] = t7.T @ t6 / sqrt(C)         -- final transpose folds away

Sharding: data-parallel over batch (32 items -> 8 cores x 4 items). Params
replicated. Host ships X in natural [C, HW] bf16 (c-contraction matmuls + sin
path) and pre-transposed [HW, C] bf16 (hw-contraction Gram matmul).

Schedule: per item the PE stream is  B(Gram) -> A(conv+max) with the four
C matmul stages interleaved -> D(out).  The elementwise sin prep runs as
whole-tile ops one item ahead: GPSIMD computes u = x + M4, DVE does the
(u - M4) - x fixup (walrus rejects STT on GPSIMD), ACT evaluates sin in
quarter-tiles ordered h0-first to match A's oh-outer loop.  All matmuls are
bf16; output is written bf16 and upcast on the host (tolerance 2e-2, measured
4.7e-3).  PSUM: pt3/po chunks are paired two-to-a-tile across two banks
([CH, 1024] tiles, chunk at each bank head) so one DVE max / ACT copy drains
two banks per op; 2 shared double-bank slots + 2 Gram accumulator banks + 2
C-transient banks = 8.  Within each pair the matmuls go kh-outer so one
LDWEIGHTS covers two chunk matmuls.  DMA: x loads + output stores ride the
qSP HWDGE ring, xt loads go through GPSIMD SWDGE as 1024-row packed
transfers, so the ACT ring stays free for compute.  Emission order keeps
loads(it) ahead of compute(it-1) so each ring's FIFO always has the next
item's loads before the previous item's stores.
"""

import functools
import math

import numpy as np

C = 256
CH = 128  # half of C; SBUF/PSUM partition count
H = W = 56
HW = H * W  # 3136
G = 8
CG = C // G
N_TOTAL = 32
N_CORES = 8
NITEM = N_TOTAL // N_CORES  # items per core
CHUNK = 392  # free-dim chunk for [C, HW] passes; 8 * 392 = 3136
NCHUNKS = HW // CHUNK
PAIRW = 2 * CHUNK  # two chunks share one [CH, 1024] two-bank PSUM tile
NPAIR = NCHUNKS // 2
BANKF = 512  # fp32 elements per PSUM bank
GTROWS = 128  # xT tile rows for the Gram pass
XTPACK = 8  # xT 128-row blocks packed per load-DMA
NXTFULL = HW // (GTROWS * XTPACK)  # 3 packed loads of 1024 rows
XTTAIL = HW - NXTFULL * GTROWS * XTPACK  # 64 tail rows
MAGIC4 = 50331648.0  # 1.5*2^25: ulp 4 -> RNE(x + MAGIC4) rounds x to mult of 4
SINPIECE = HW // 2  # sin evaluated in two pieces per half for earlier firing


def _build_program(repeat=1, phases="ABCD", split="b", xdt="f32r"):
    import concourse.mybir as mybir
    import concourse.tile as tile
    from concourse import bacc

    from concourse.alu_op_type import AluOpType

    f32 = mybir.dt.float32
    bf16 = mybir.dt.bfloat16
    SIN = mybir.ActivationFunctionType.Sin

    nc = bacc.Bacc("TRN2", target_bir_lowering=False, debug=False,
                   num_devices=N_CORES)

    x_d = nc.dram_tensor("x", [NITEM, C, HW], bf16, kind="ExternalInput").ap()
    xt_d = nc.dram_tensor("xt", [NITEM, HW, C], bf16, kind="ExternalInput").ap()
    w3t_d = nc.dram_tensor("w3t", [2, CH, C], bf16, kind="ExternalInput").ap()
    b1t_d = nc.dram_tensor("b1t", [2, CH, C], bf16, kind="ExternalInput").ap()
    b2t_d = nc.dram_tensor("b2t", [2, CH, C], bf16, kind="ExternalInput").ap()
    p7_d = nc.dram_tensor("p7", [2, CH, C], f32, kind="ExternalInput").ap()
    out_d = nc.dram_tensor("out", [NITEM, C, HW], bf16,
                           kind="ExternalOutput").ap()

    from contextlib import ExitStack

    with tile.TileContext(nc) as tc, ExitStack() as ctx:
        wpool = ctx.enter_context(tc.tile_pool(name="w", bufs=1))
        xpool = ctx.enter_context(tc.tile_pool(name="x", bufs=2))
        xtpool = ctx.enter_context(tc.tile_pool(name="xt", bufs=3))
        ttpool = ctx.enter_context(tc.tile_pool(name="tt", bufs=2))
        t4pool = ctx.enter_context(tc.tile_pool(name="t4", bufs=2))
        t6pool = ctx.enter_context(tc.tile_pool(name="t6", bufs=2))
        gsbp = ctx.enter_context(tc.tile_pool(name="gsb", bufs=2))
        t7p = ctx.enter_context(tc.tile_pool(name="t7", bufs=2))
        outp = ctx.enter_context(tc.tile_pool(name="outs", bufs=2))
        # PSUM: 8 banks = three double-bank pt3/po slots (6) + 2 banks shared
        # by the Gram accumulators and (after their evacuation frees the
        # slots, right when C begins) the C transients.
        ps_sh = ctx.enter_context(tc.tile_pool(name="ps", bufs=3, space="PSUM"))
        ps_g = ctx.enter_context(tc.tile_pool(name="psg", bufs=2, space="PSUM"))
        ps_c = ps_g

        # replicated params
        w3t_sb = [wpool.tile([CH, C], bf16, tag=f"w3t{k}", name=f"w3t{k}")
                  for k in range(2)]
        b1t_sb = [wpool.tile([CH, C], bf16, tag=f"b1t{k}", name=f"b1t{k}")
                  for k in range(2)]
        b2t_sb = [wpool.tile([CH, C], bf16, tag=f"b2t{k}", name=f"b2t{k}")
                  for k in range(2)]
        p7_sb = [wpool.tile([CH, C], f32, tag=f"p7{k}", name=f"p7{k}")
                 for k in range(2)]
        for k in range(2):
            nc.sync.dma_start(out=w3t_sb[k][:], in_=w3t_d[k])
            nc.sync.dma_start(out=b1t_sb[k][:], in_=b1t_d[k])
            nc.sync.dma_start(out=b2t_sb[k][:], in_=b2t_d[k])
            nc.sync.dma_start(out=p7_sb[k][:], in_=p7_d[k])

        half = lambda ap, h: ap[:, h * CH:(h + 1) * CH]

        state = {}

        def emit_loads(it):
            xa = []
            for h in range(2):
                t = xpool.tile([CH, HW], bf16, tag=f"xa{h}",
                               name=f"xa_{it}_{h}")
                nc.sync.dma_start(out=t[:], in_=x_d[it, h * CH:(h + 1) * CH, :])
                xa.append(t)
            xts, xt_s = [], None
            if "noB" not in phases:
                for tj in range(NXTFULL):
                    t = xtpool.tile([GTROWS, XTPACK, C], bf16, tag="xtt",
                                    name=f"xtt_{it}_{tj}")
                    src = xt_d[it,
                               tj * GTROWS * XTPACK:(tj + 1) * GTROWS * XTPACK, :]
                    nc.gpsimd.dma_start(
                        out=t[:], in_=src.rearrange("(b p) c -> p b c", p=GTROWS))
                    xts.append(t)
                xt_s = xtpool.tile([GTROWS, C], bf16, tag="xts",
                                   name=f"xts_{it}")
                nc.gpsimd.dma_start(out=xt_s[:XTTAIL, :],
                                    in_=xt_d[it, HW - XTTAIL:, :])
            state[it] = dict(xa=xa, xts=xts, xt_s=xt_s)

        def emit_prepass(it):
            st = state[it]
            if "noEl" in phases:
                st["t4"] = None
                return
            xa = st["xa"]
            tts, t4s = [], []
            for h in range(2):
                tt = ttpool.tile([CH, HW], f32, tag=f"tt{h}",
                                 name=f"tt_{it}_{h}")
                t4 = t4pool.tile([CH, HW], bf16, tag=f"t4{h}",
                                 name=f"t4_{it}_{h}")
                tts.append(tt)
                t4s.append(t4)
            # u = RNE(x + M4) == 4*round(x/4) + M4 (ulp 4); in-place fixup
            # tt = (u - M4) - x == -y;  t4 = sin(-pi/2 * -y) = sin(pi/2 x)
            nc.gpsimd.tensor_scalar(tts[0][:], xa[0][:], MAGIC4, None,
                                    AluOpType.add)
            nc.gpsimd.tensor_scalar(tts[1][:], xa[1][:], MAGIC4, None,
                                    AluOpType.add)
            nc.vector.scalar_tensor_tensor(
                tts[0][:], tts[0][:], MAGIC4, xa[0][:],
                AluOpType.subtract, AluOpType.subtract)
            nc.vector.scalar_tensor_tensor(
                tts[1][:], tts[1][:], MAGIC4, xa[1][:],
                AluOpType.subtract, AluOpType.subtract)
            # one sin per half; A's loop is oh-outer so h0 completes first
            for h in range(2):
                nc.scalar.activation(t4s[h][:], tts[h][:], SIN,
                                     scale=-math.pi / 2)
            st["t4"] = t4s

        def emit_compute(it):
            st = state[it]
            xa, xts, xt_s, t4s = st["xa"], st["xts"], st["xt_s"], st["t4"]

            # ---- B: G = X @ X.T (bf16), contraction over HW ----
            if "noB" not in phases:
                psG = [ps_g.tile([CH, C], f32, tag="psg", name=f"psg_{it}_{h}")
                       for h in range(2)]
                ktile = 0
                nktiles = NXTFULL * XTPACK + (1 if XTTAIL else 0)
                for tj in range(NXTFULL):
                    for b in range(XTPACK):
                        for eh in range(2):
                            nc.tensor.matmul(
                                psG[eh][:],
                                xts[tj][:, b, eh * CH:(eh + 1) * CH],
                                xts[tj][:, b, :],
                                start=(ktile == 0),
                                stop=(ktile == nktiles - 1))
                        ktile += 1
                if XTTAIL:
                    for eh in range(2):
                        nc.tensor.matmul(
                            psG[eh][:], xt_s[:XTTAIL, eh * CH:(eh + 1) * CH],
                            xt_s[:XTTAIL, :],
                            start=(ktile == 0), stop=True)
                    ktile += 1

                # Gram evacuation (frees the psg banks for the next item)
                g_sb = [gsbp.tile([CH, C], bf16, tag=f"g{h}",
                                  name=f"g_{it}_{h}") for h in range(2)]
                for eh in range(2):
                    nc.scalar.copy(g_sb[eh][:], psG[eh][:])

            # ---- A: t3 = Wp3 @ X; t6 = max(t3, t4); C interleaved ----
            t6 = [t6pool.tile([CH, HW], bf16, tag=f"t6h{h}",
                              name=f"t6_{it}_{h}") for h in range(2)]
            gb_sb = [gsbp.tile([CH, C], bf16, tag=f"gb{h}",
                               name=f"gb_{it}_{h}") for h in range(2)]
            t7_sb = [t7p.tile([CH, C], bf16, tag=f"t7{h}",
                              name=f"t7_{it}_{h}") for h in range(2)]

            def emit_c_piece(step):
                # four 2-matmul stages of C spread across A's chunk loop
                if step == 0:  # pgb0 = (G @ B2t)[0]
                    pgb = ps_c.tile([CH, C], f32, tag="psg",
                                    name=f"pgb_{it}_0")
                    nc.tensor.matmul(pgb[:], half(g_sb[0], 0)[:],
                                     b2t_sb[0][:], start=True, stop=False)
                    nc.tensor.matmul(pgb[:], half(g_sb[1], 0)[:],
                                     b2t_sb[1][:], start=False, stop=True)
                    nc.scalar.copy(gb_sb[0][:], pgb[:])
                elif step == 1:  # pgb1
                    pgb = ps_c.tile([CH, C], f32, tag="psg",
                                    name=f"pgb_{it}_1")
                    nc.tensor.matmul(pgb[:], half(g_sb[0], 1)[:],
                                     b2t_sb[0][:], start=True, stop=False)
                    nc.tensor.matmul(pgb[:], half(g_sb[1], 1)[:],
                                     b2t_sb[1][:], start=False, stop=True)
                    nc.scalar.copy(gb_sb[1][:], pgb[:])
                elif step in (2, 3):  # t5 halves + t7 = p7 * t5
                    ch = step - 2
                    pt5 = ps_c.tile([CH, C], f32, tag="psg",
                                    name=f"pt5_{it}_{ch}")
                    nc.tensor.matmul(pt5[:], half(b1t_sb[0], ch)[:],
                                     gb_sb[0][:], start=True, stop=False)
                    nc.tensor.matmul(pt5[:], half(b1t_sb[1], ch)[:],
                                     gb_sb[1][:], start=False, stop=True)
                    nc.vector.tensor_mul(t7_sb[ch][:], pt5[:], p7_sb[ch][:])

            def pairview(ap2d):
                # [CH, 2*BANKF] -> [CH, 2, CHUNK]: chunk in each bank's head
                return ap2d.rearrange("p (b c) -> p b c", b=2)[:, :, 0:CHUNK]

            def sbview(big, pj):
                # contiguous [CH, PAIRW] slice as [CH, 2, CHUNK]
                return big[:, pj * PAIRW:(pj + 1) * PAIRW].rearrange(
                    "p (b c) -> p b c", b=2)

            if "noB" in phases:
                for ch in range(2):
                    nc.vector.tensor_copy(t7_sb[ch][:], p7_sb[ch][:])

            c_sched = {1: 0, 3: 1, 6: 2, 7: 3}  # after A-pair-unit -> C step
            unit = 0
            for oh in range(2):
                for pj in range(NPAIR):
                    j0 = 2 * pj
                    js0 = slice(j0 * CHUNK, (j0 + 1) * CHUNK)
                    js1 = slice((j0 + 1) * CHUNK, (j0 + 2) * CHUNK)
                    pt3 = ps_sh.tile([CH, 2 * BANKF], f32, tag="ps",
                                     name=f"pt3_{it}_{oh}_{pj}")
                    # kh-outer: one weight load covers both chunk matmuls
                    nc.tensor.matmul(pt3[:, 0:CHUNK], half(w3t_sb[0], oh)[:],
                                     xa[0][:, js0], start=True, stop=False)
                    nc.tensor.matmul(pt3[:, BANKF:BANKF + CHUNK],
                                     half(w3t_sb[0], oh)[:],
                                     xa[0][:, js1], start=True, stop=False)
                    nc.tensor.matmul(pt3[:, 0:CHUNK], half(w3t_sb[1], oh)[:],
                                     xa[1][:, js0], start=False, stop=True)
                    nc.tensor.matmul(pt3[:, BANKF:BANKF + CHUNK],
                                     half(w3t_sb[1], oh)[:],
                                     xa[1][:, js1], start=False, stop=True)
                    if "noEl" in phases:
                        nc.vector.tensor_copy(sbview(t6[oh], pj),
                                              pairview(pt3[:]))
                    else:
                        nc.vector.tensor_max(sbview(t6[oh], pj),
                                             pairview(pt3[:]),
                                             sbview(t4s[oh], pj))
                    unit += 1
                    if unit in c_sched and "noB" not in phases:
                        emit_c_piece(c_sched[unit])

            # ---- D: out = t7.T @ t6, bf16 staging, one store per half ----
            if "noD" in phases:
                nc.sync.dma_start(out=out_d[it, 0:CH, :], in_=t6[0][:])
                nc.sync.dma_start(out=out_d[it, CH:C, :], in_=t6[1][:])
                return
            os_t = [outp.tile([CH, HW], bf16, tag=f"os{dh}",
                              name=f"os_{it}_{dh}") for dh in range(2)]
            for pj in range(NPAIR):
                j0 = 2 * pj
                js0 = slice(j0 * CHUNK, (j0 + 1) * CHUNK)
                js1 = slice((j0 + 1) * CHUNK, (j0 + 2) * CHUNK)
                for dh in range(2):
                    po = ps_sh.tile([CH, 2 * BANKF], f32, tag="ps",
                                    name=f"po_{it}_{pj}_{dh}")
                    # kh-outer: one weight load covers both chunk matmuls
                    nc.tensor.matmul(po[:, 0:CHUNK], half(t7_sb[0], dh)[:],
                                     t6[0][:, js0], start=True, stop=False)
                    nc.tensor.matmul(po[:, BANKF:BANKF + CHUNK],
                                     half(t7_sb[0], dh)[:],
                                     t6[0][:, js1], start=True, stop=False)
                    nc.tensor.matmul(po[:, 0:CHUNK], half(t7_sb[1], dh)[:],
                                     t6[1][:, js0], start=False, stop=True)
                    nc.tensor.matmul(po[:, BANKF:BANKF + CHUNK],
                                     half(t7_sb[1], dh)[:],
                                     t6[1][:, js1], start=False, stop=True)
                    nc.scalar.copy(sbview(os_t[dh], pj), pairview(po[:]))
            nc.sync.dma_start(out=out_d[it, 0:CH, :], in_=os_t[0][:])
            nc.sync.dma_start(out=out_d[it, CH:C, :], in_=os_t[1][:])

        for rep in range(repeat):
            for it in range(NITEM):
                key = it  # state keyed per item; tiles rotate via pools
                emit_loads(it)
                if it > 0:
                    emit_compute(it - 1)
                emit_prepass(it)
            emit_compute(NITEM - 1)

    nc.compile()
    return nc


@functools.lru_cache(maxsize=16)
def _get_runner(repeat=1, phases="ABCD", split="b", xdt="f32r"):
    """Build the Bass program once and return a reusable executor."""
    import jax
    import numpy as _np
    from jax.sharding import Mesh, PartitionSpec
    from jax.experimental.shard_map import shard_map
    import concourse.mybir as mybir
    from concourse import bass2jax

    nc = _build_program(repeat, phases, split, xdt)
    bass2jax.install_neuronx_cc_hook()

    partition_name = (nc.partition_id_tensor.name
                      if nc.partition_id_tensor else None)
    in_names, out_names, out_avals, zero_outs = [], [], [], []
    for alloc in nc.m.functions[0].allocations:
        if not isinstance(alloc, mybir.MemoryLocationSet):
            continue
        name = alloc.memorylocations[0].name
        if alloc.kind == "ExternalInput":
            if name != partition_name:
                in_names.append(name)
        elif alloc.kind == "ExternalOutput":
            shape = tuple(alloc.tensor_shape)
            dtype = mybir.dt.np(alloc.dtype)
            out_names.append(name)
            out_avals.append(jax.core.ShapedArray(shape, dtype))
            zero_outs.append(_np.zeros(shape, dtype))
    n_params = len(in_names)
    all_in_names = list(in_names) + list(out_names)
    if partition_name is not None:
        all_in_names.append(partition_name)

    def _body(*args):
        operands = list(args)
        if partition_name is not None:
            operands.append(bass2jax.partition_id_tensor())
        outs = bass2jax._bass_exec_p.bind(
            *operands,
            out_avals=tuple(out_avals),
            in_names=tuple(all_in_names),
            out_names=tuple(out_names),
            lowering_input_output_aliases=(),
            sim_require_finite=True,
            sim_require_nnan=True,
            nc=nc,
        )
        return tuple(outs)

    devices = jax.devices()[:N_CORES]
    mesh = Mesh(_np.asarray(devices), ("core",))
    n_outs = len(out_names)
    in_specs = (PartitionSpec("core"),) * (n_params + n_outs)
    out_specs = (PartitionSpec("core"),) * n_outs
    sharded = jax.jit(
        shard_map(_body, mesh=mesh, in_specs=in_specs, out_specs=out_specs,
                  check_rep=False),
        keep_unused=True,
    )
    concat_zeros = [
        _np.zeros((N_CORES * z.shape[0], *z.shape[1:]), z.dtype)
        for z in zero_outs
    ]

    def run(per_core_maps):
        concat_in = [
            _np.concatenate([per_core_maps[c][name] for c in range(N_CORES)],
                            axis=0)
            for name in in_names
        ]
        out_arrs = sharded(*concat_in, *concat_zeros)
        return [
            {name: _np.asarray(out_arrs[i]).reshape(
                N_CORES, *out_avals[i].shape)[c]
             for i, name in enumerate(out_names)}
            for c in range(N_CORES)
        ]

    run.in_names = in_names
    run.sharded = sharded
    run.concat_zeros = concat_zeros
    run.out_avals = out_avals
    run.out_names = out_names
    return run


def _prepare_inputs(x, Wp3, bp3, Wg1, bg1, Wg2, bg2, p7w, xdt="bf16"):
    import ml_dtypes

    for nm, b in (("bp3", bp3), ("bg1", bg1), ("bg2", bg2)):
        if np.abs(np.asarray(b)).max() != 0.0:
            raise NotImplementedError(f"nonzero bias {nm} not supported")

    x = np.ascontiguousarray(np.asarray(x, dtype=np.float32)).reshape(
        N_TOTAL, C, HW)
    xt = np.ascontiguousarray(
        x.transpose(0, 2, 1)).astype(ml_dtypes.bfloat16)
    x = x.astype(ml_dtypes.bfloat16)

    w3t = np.ascontiguousarray(np.asarray(Wp3, np.float32).T).reshape(
        2, CH, C).astype(ml_dtypes.bfloat16)

    def blockdiag(Wg):
        B = np.zeros((C, C), np.float32)
        for g in range(G):
            B[g * CG:(g + 1) * CG, g * CG:(g + 1) * CG] = Wg[g]
        return B

    b1t = np.ascontiguousarray(blockdiag(np.asarray(Wg1, np.float32)).T
                               ).reshape(2, CH, C).astype(ml_dtypes.bfloat16)
    b2t = np.ascontiguousarray(
        blockdiag(np.asarray(Wg2, np.float32)).T / math.sqrt(HW)
    ).reshape(2, CH, C).astype(ml_dtypes.bfloat16)
    p7 = np.ascontiguousarray(
        np.asarray(p7w, np.float32)[0] / math.sqrt(C)).reshape(2, CH, C)

    maps = []
    for c in range(N_CORES):
        sl = slice(c * NITEM, (c + 1) * NITEM)
        maps.append({
            "x": x[sl], "xt": xt[sl],
            "w3t": w3t, "b1t": b1t, "b2t": b2t, "p7": p7,
        })
    return maps


def kernel(x, Wp3, bp3, Wg1, bg1, Wg2, bg2, p7w):
    runner = _get_runner(1, "ABCD", "f", "bf16")
    maps = _prepare_inputs(x, Wp3, bp3, Wg1, bg1, Wg2, bg2, p7w)
    results = runner(maps)
    out = np.concatenate([results[c]["out"] for c in range(N_CORES)], axis=0)
    return out.astype(np.float32).reshape(N_TOTAL, C, H, W)
